# revision 8
# baseline (speedup 1.0000x reference)
"""Trainium2 Bass kernel for the DGCNN-style message-passing block.

Math (per batch b, data-parallel over 8 cores), with all inference-BNs folded
host-side into row/column scalings of the weights (X1 = s_f*x1, X2 = s_f*x2):
    proj = x @ Wp^T                       (bf16, stays in SBUF)
    m[i] = max_k proj[knn[i,k]]           (edge maxpool: max_k(f_j-f_i) = m_i - proj_i)
    X1 = m@Wm1^T + x@Wx1^T + b1c          (Wm1 = s_f*Wlp, Wx1 = -s_f*Wlp@Wp)
    X2 = x@Wx2^T + b2c                    (Wx2 = s_f*Wgp)
    f' = X1 + X2   (= s_f*f; 1/s_f folded into W1'' columns)
    h  = f'@W1''^T + bh;  a = sigmoid(h@W2'^T + ba)
    out = (a*(X1-X2) + t_f) + X2

The KNN max-gather runs as SBUF-source transposed dma_gather custom ops:
proj lives in SBUF node-major ([128, 32 stripes, 256ch] bf16; node n at
partition n%128, rank n//128), and each gather pulls 2048 rows (4 neighbor
slots x 512 nodes, k-major) transposed into feature-major [ch, (k,node)]
tiles, so no DRAM round-trip and no PE transposes are needed.  A 4-level DVE
max tree reduces k; all matmuls are bf16 with weights stationary.
"""

import numpy as np
import ml_dtypes

import concourse.bass as bass
import concourse.mybir as mybir
import concourse.tile as tile
from concourse import bacc
from concourse.bass_utils import run_bass_kernel_spmd

F32 = mybir.dt.float32
BF16 = mybir.dt.bfloat16
I16 = mybir.dt.int16

B, N, K, C = 8, 4096, 16, 256
P = 128
CK = C // P          # 2 channel chunks
NT = N // P          # 32 proj stripes
NO = 8               # node octants
U = N // NO          # 512 nodes per octant
G = 2048             # indices per gather
GPO = K * U // G     # 4 gathers per octant (4 k-slots each)
EPS = 1e-5

AF = mybir.ActivationFunctionType
ALU = mybir.AluOpType


def build_bass(n_cores: int = 8, reps: int = 1):
    nc = bacc.Bacc(
        "TRN2",
        target_bir_lowering=False,
        debug=False,
        enable_asserts=False,
        num_devices=n_cores,
        num_swdge_queues=4,
        dynamic_dma_scratch_size=32768,
    )

    xT = nc.dram_tensor("xT", [C, N], BF16, kind="ExternalInput").ap()
    knn_i = nc.dram_tensor("knn_i", [P, NO * GPO * (G // 16)], I16, kind="ExternalInput").ap()
    # packed bf16 weights: block (w_i, kc) = W^T[kc*128:(kc+1)*128, :] at
    # [:, (w_i*CK+kc)*C : +C]; order: wp, wx1, wx2, wm1, w1'', w2'
    wb = nc.dram_tensor("wb", [P, 6 * CK * C], BF16, kind="ExternalInput").ap()
    bias = nc.dram_tensor("bias", [P, 10], F32, kind="ExternalInput").ap()
    outT = nc.dram_tensor("outT", [C, N], BF16, kind="ExternalOutput").ap()

    with tile.TileContext(nc) as tc:
        for _ in range(reps):
            kernel_body(tc, xT, knn_i, wb, bias, outT)
    nc.compile()
    return nc


def kernel_body(tc, xT, knn_i, wb, bias, outT):
    nc = tc.nc

    with (
        tc.tile_pool(name="const", bufs=1) as cpool,
        tc.tile_pool(name="projp", bufs=1) as projp,
        tc.tile_pool(name="gat", bufs=1) as gat,
        tc.tile_pool(name="mtp", bufs=1) as mtp,
        tc.tile_pool(name="units", bufs=2) as up,
        tc.tile_pool(name="outp", bufs=2) as outp,
        tc.tile_pool(name="psP", bufs=2, space="PSUM") as psP,
        tc.tile_pool(name="psX", bufs=4, space="PSUM") as psX,
        tc.tile_pool(name="psH", bufs=2, space="PSUM") as psH,
    ):
        # ---- constants / inputs to SBUF (order = first-use order) ----
        wb_sb = cpool.tile([P, 6 * CK * C], BF16)
        nc.sync.dma_start(wb_sb[:, 0:CK * C], wb[:, 0:CK * C])          # wp first

        def wslice(w_i):
            return [wb_sb[:, (w_i * CK + kc) * C:(w_i * CK + kc + 1) * C]
                    for kc in range(CK)]

        wp_sb, wx1_sb, wx2_sb, wm1_sb, w1_sb, w2_sb = (wslice(i) for i in range(6))

        # xt[p, kc, n]: x^T in bf16, loaded per octant for early proj start
        xt = cpool.tile([P, CK, N], BF16)
        xTr = xT.rearrange("(kc p) n -> p kc n", p=P)
        for o in range(NO):
            nc.sync.dma_start(xt[:, :, o * U:(o + 1) * U],
                              xTr[:, :, o * U:(o + 1) * U])

        kidx = cpool.tile([P, NO * GPO, G // 16], I16)
        nc.sync.dma_start(kidx[:], knn_i[:].rearrange("p (g c) -> p g c", c=G // 16))

        nc.sync.dma_start(wb_sb[:, CK * C:], wb[:, CK * C:])

        bias_sb = cpool.tile([P, 10], F32)
        nc.sync.dma_start(bias_sb[:], bias[:])

        # ---- phase 1: proj, node-major bf16, stays in SBUF ----
        # proj_sb[p, t, c] = proj[node t*128+p, c]; two stripes share one PSUM
        # bank so each evac moves 512 elems; evacs alternate Act/DVE.
        proj_sb = projp.tile([P, NT, C], BF16)
        for tp in range(NT // 2):
            ps = psP.tile([P, 2, C], F32, name="ps_proj", tag="ps_proj")
            for half in range(2):
                t = 2 * tp + half
                nc.tensor.matmul(ps[:, half, :], lhsT=xt[:, 0, t * P:(t + 1) * P],
                                 rhs=wp_sb[0], start=half == 0 and True, stop=False,
                                 skip_group_check=True)
                nc.tensor.matmul(ps[:, half, :], lhsT=xt[:, 1, t * P:(t + 1) * P],
                                 rhs=wp_sb[1], start=False, stop=half == 1,
                                 skip_group_check=True)
            dst = proj_sb[:, 2 * tp:2 * tp + 2, :]
            if tp % 2 == 0:
                nc.scalar.activation(dst, ps[:], AF.Copy)
            else:
                nc.vector.tensor_copy(dst, ps[:])

        proj_flat = proj_sb[:].rearrange("p t c -> p (t c)")

        # ---- phases 2-4: per-octant pipeline ----
        # Three-stage software pipeline so no engine queue ever parks on a
        # cross-engine round-trip:
        #   A(o)  = gathers + max tree (DVE, one level on Pool)
        #   B1(o) = PE matmuls + Act evacs (x1/x2/h/a)
        #   B2(o) = DVE combines (dd, g2, out) + out DMA, one octant later
        mts = [None] * NO
        units = [None] * NO

        def phase_a(o):
            go = []
            for g in range(GPO):
                gt = gat.tile([P, CK, G], BF16, name=f"go{o}_{g}", tag="go", bufs=8)
                nc.gpsimd.dma_gather(
                    out_ap=gt[:],
                    in_ap=proj_flat,
                    idxs_ap=kidx[:, o * GPO + g, :],
                    num_idxs=G,
                    num_idxs_reg=G,
                    elem_size=C,
                    transpose=True,
                    single_packet=False,
                    sbuf_tokens_per_rank=P,
                    sbuf_free_dim_per_rank=C * 2,
                    queue_num=g % 4,
                )
                go.append(gt)

            t01 = mtp.tile([P, CK, G], BF16, name="t01", tag="t01", bufs=2)
            nc.vector.tensor_tensor(out=t01[:], in0=go[0][:], in1=go[1][:],
                                    op=ALU.max)
            t0123 = mtp.tile([P, CK, G], BF16, name="t0123", tag="t0123", bufs=2)
            nc.vector.tensor_tensor(out=t0123[:], in0=go[2][:], in1=go[3][:],
                                    op=ALU.max)
            nc.vector.tensor_tensor(out=t0123[:], in0=t0123[:], in1=t01[:],
                                    op=ALU.max)
            m3 = mtp.tile([P, CK, 2 * U], BF16, name="m3", tag="m3", bufs=2)
            nc.gpsimd.tensor_tensor(out=m3[:], in0=t0123[:, :, :2 * U],
                                    in1=t0123[:, :, 2 * U:], op=ALU.max)
            mt = mtp.tile([P, CK, U], BF16, name="mt", tag="mt", bufs=3)
            nc.vector.tensor_tensor(out=mt[:], in0=m3[:, :, :U],
                                    in1=m3[:, :, U:], op=ALU.max)
            mts[o] = mt

        def phase_b1(o):
            mt = mts[o]
            x1_sb = up.tile([P, CK, U], BF16, name="x1", tag="x1", bufs=3)
            x2_sb = up.tile([P, CK, U], BF16, name="x2", tag="x2", bufs=3)
            h_sb = up.tile([P, CK, U], BF16, name="h", tag="h", bufs=3)
            a_sb = up.tile([P, CK, U], BF16, name="a", tag="a", bufs=3)
            units[o] = (x1_sb, x2_sb, a_sb)

            xo = [xt[:, kc, o * U:(o + 1) * U] for kc in range(CK)]

            for mc in range(CK):
                ps1 = psX.tile([P, U], F32, name="ps_x1", tag="ps_x1", bufs=2)
                for kc in range(CK):
                    nc.tensor.matmul(ps1[:], lhsT=wm1_sb[kc][:, mc * P:(mc + 1) * P],
                                     rhs=mt[:, kc, :], start=kc == 0, stop=False,
                                     skip_group_check=True)
                for kc in range(CK):
                    nc.tensor.matmul(ps1[:], lhsT=wx1_sb[kc][:, mc * P:(mc + 1) * P],
                                     rhs=xo[kc], start=False, stop=kc == CK - 1,
                                     skip_group_check=True)
                nc.scalar.activation(x1_sb[:, mc, :], ps1[:], AF.Identity,
                                     bias=bias_sb[:, 0 + mc:1 + mc], scale=1.0)
                ps2 = psX.tile([P, U], F32, name="ps_x2", tag="ps_x2", bufs=2)
                for kc in range(CK):
                    nc.tensor.matmul(ps2[:], lhsT=wx2_sb[kc][:, mc * P:(mc + 1) * P],
                                     rhs=xo[kc], start=kc == 0, stop=kc == CK - 1,
                                     skip_group_check=True)
                nc.scalar.activation(x2_sb[:, mc, :], ps2[:], AF.Identity,
                                     bias=bias_sb[:, 2 + mc:3 + mc], scale=1.0)

            # h = f@W1'' with f = X1 + X2 distributed into the matmul
            for mc in range(CK):
                psh = psH.tile([P, U], F32, name="ps_h", tag="ps_h", bufs=1)
                for kc in range(CK):
                    nc.tensor.matmul(psh[:], lhsT=w1_sb[kc][:, mc * P:(mc + 1) * P],
                                     rhs=x1_sb[:, kc, :], start=kc == 0, stop=False,
                                     skip_group_check=True)
                for kc in range(CK):
                    nc.tensor.matmul(psh[:], lhsT=w1_sb[kc][:, mc * P:(mc + 1) * P],
                                     rhs=x2_sb[:, kc, :], start=False,
                                     stop=kc == CK - 1, skip_group_check=True)
                nc.scalar.activation(h_sb[:, mc, :], psh[:], AF.Identity,
                                     bias=bias_sb[:, 4 + mc:5 + mc], scale=1.0)
            for mc in range(CK):
                psa = psH.tile([P, U], F32, name="ps_a", tag="ps_a", bufs=1)
                for kc in range(CK):
                    nc.tensor.matmul(psa[:], lhsT=w2_sb[kc][:, mc * P:(mc + 1) * P],
                                     rhs=h_sb[:, kc, :], start=kc == 0,
                                     stop=kc == CK - 1, skip_group_check=True)
                nc.scalar.activation(a_sb[:, mc, :], psa[:], AF.Sigmoid,
                                     bias=bias_sb[:, 6 + mc:7 + mc], scale=1.0)

        def phase_b2(o):
            x1_sb, x2_sb, a_sb = units[o]
            dd_sb = up.tile([P, CK, U], BF16, name="dd", tag="dd", bufs=2)
            g2_sb = up.tile([P, CK, U], BF16, name="g2", tag="g2", bufs=2)
            nc.vector.tensor_tensor(out=dd_sb[:], in0=x1_sb[:], in1=x2_sb[:],
                                    op=ALU.subtract)
            nc.vector.tensor_tensor(out=g2_sb[:], in0=a_sb[:], in1=dd_sb[:],
                                    op=ALU.mult)
            # out = (g2 + t_f) + X2, per out-channel chunk (t_f is per-partition)
            for mc in range(CK):
                ot = outp.tile([P, U], BF16, name="ot", tag="ot")
                nc.vector.scalar_tensor_tensor(
                    out=ot[:], in0=g2_sb[:, mc, :],
                    scalar=bias_sb[:, 8 + mc:9 + mc],
                    in1=x2_sb[:, mc, :], op0=ALU.add, op1=ALU.add)
                nc.sync.dma_start(outT[mc * P:(mc + 1) * P, o * U:(o + 1) * U], ot[:])

        phase_a(0)
        phase_a(1)
        phase_b1(0)
        for o in range(2, NO):
            phase_a(o)
            phase_b1(o - 1)
            phase_b2(o - 2)
        phase_b1(NO - 1)
        phase_b2(NO - 2)
        phase_b2(NO - 1)


# ---------------- host side ----------------

def _fold(proj_W, local_W, glob_W, aff_W1, aff_b1, aff_W2, aff_b2,
          bn_local, bn_glob, bn_aff1, bn_aff2, bn_final):
    f32 = np.float32

    def bn_st(p):
        p = np.asarray(p, f32)
        g, b, m, v = p
        s = g / np.sqrt(v + EPS)
        return s.astype(f32), (b - m * s).astype(f32)

    Wp = np.asarray(proj_W, f32)
    s_l, t_l = bn_st(bn_local)
    s_g, t_g = bn_st(bn_glob)
    s_1, t_1 = bn_st(bn_aff1)
    s_2, t_2 = bn_st(bn_aff2)
    s_f, t_f = bn_st(bn_final)

    Wlp = s_l[:, None] * np.asarray(local_W, f32)
    Wgp = s_g[:, None] * np.asarray(glob_W, f32)

    wp = Wp
    wx1 = -(s_f[:, None] * (Wlp @ Wp))
    wx2 = s_f[:, None] * Wgp
    wm1 = s_f[:, None] * Wlp
    w1 = (s_1[:, None] * np.asarray(aff_W1, f32)) * (1.0 / s_f)[None, :]
    w2 = s_2[:, None] * np.asarray(aff_W2, f32)

    out = np.zeros((P, 6 * CK * C), ml_dtypes.bfloat16)
    for w_i, m in enumerate((wp, wx1, wx2, wm1, w1, w2)):
        mT = m.T
        for kc in range(CK):
            out[:, (w_i * CK + kc) * C:(w_i * CK + kc + 1) * C] = \
                mT[kc * P:(kc + 1) * P, :].astype(ml_dtypes.bfloat16)

    b1c = s_f * t_l
    b2c = s_f * t_g
    bh = s_1 * np.asarray(aff_b1, f32) + t_1
    ba = s_2 * np.asarray(aff_b2, f32) + t_2
    bias = np.zeros((P, 10), f32)
    for j, tt in enumerate((b1c, b2c, bh, ba, t_f)):
        for mc in range(CK):
            bias[:, 2 * j + mc] = tt[mc * P:(mc + 1) * P]
    return {"wb": out, "bias": bias}


def _pack_idx(knn_b):
    """knn[b] [N, K] int -> [128, NO*GPO*(G//16)] int16 wrapped gather idxs.

    Per (octant o, gather g): 2048 idxs k-major (j = k'*U + i, k' in [0,4)),
    wrapped [16, G//16] with [w, c] = idx[c*16+w], replicated 8x over
    partition groups."""
    cols = []
    for o in range(NO):
        blk = knn_b[o * U:(o + 1) * U, :]          # [U, K]
        for g in range(GPO):
            arr = blk[:, 4 * g:4 * g + 4].T.reshape(-1)   # [4*U] k-major
            wrapped = arr.reshape(G // 16, 16).T           # [16, G//16]
            cols.append(np.tile(wrapped, (8, 1)))          # [128, G//16]
    return np.concatenate(cols, axis=1).astype(np.int16)


_NC_CACHE = {}


def _get_nc():
    if "nc" not in _NC_CACHE:
        _NC_CACHE["nc"] = build_bass(B)
    return _NC_CACHE["nc"]


def kernel(**inputs) -> np.ndarray:
    x = np.asarray(inputs["x"], np.float32)                 # [B,N,C]
    knn = np.asarray(inputs["knn"]).astype(np.int64)        # [B,N,K]
    w = _fold(
        inputs["proj_W"], inputs["local_W"], inputs["glob_W"],
        inputs["aff_W1"], inputs["aff_b1"], inputs["aff_W2"], inputs["aff_b2"],
        inputs["bn_local"], inputs["bn_glob"], inputs["bn_aff1"],
        inputs["bn_aff2"], inputs["bn_final"],
    )

    nc = _get_nc()
    in_maps = []
    for b in range(B):
        m = {"xT": np.ascontiguousarray(x[b].T).astype(ml_dtypes.bfloat16),
             "knn_i": _pack_idx(knn[b])}
        m.update(w)
        in_maps.append(m)

    res = run_bass_kernel_spmd(nc, in_maps, core_ids=list(range(B)))
    out = np.stack([res.results[b]["outT"].astype(np.float32).T
                    for b in range(B)])
    return out


if __name__ == "__main__":
    nc = build_bass(1)
    print("built OK")


# revision 9
# speedup vs baseline: 1.0701x; 1.0701x over previous
"""Trainium2 Bass kernel for the DGCNN-style message-passing block.

Math (per batch b, data-parallel over 8 cores), with all inference-BNs folded
host-side into row/column scalings of the weights (X1 = s_f*x1, X2 = s_f*x2):
    proj = x @ Wp^T                       (bf16, stays in SBUF)
    m[i] = max_k proj[knn[i,k]]           (edge maxpool: max_k(f_j-f_i) = m_i - proj_i)
    X1 = m@Wm1^T + x@Wx1^T + b1c          (Wm1 = s_f*Wlp, Wx1 = -s_f*Wlp@Wp)
    X2 = x@Wx2^T + b2c                    (Wx2 = s_f*Wgp)
    f' = X1 + X2   (= s_f*f; 1/s_f folded into W1'' columns)
    h  = f'@W1''^T + bh;  a = sigmoid(h@W2'^T + ba)
    out = (a*(X1-X2) + t_f) + X2

The KNN max-gather runs as SBUF-source transposed dma_gather custom ops:
proj lives in SBUF node-major ([128, 32 stripes, 256ch] bf16; node n at
partition n%128, rank n//128), and each gather pulls 2048 rows (4 neighbor
slots x 512 nodes, k-major) transposed into feature-major [ch, (k,node)]
tiles, so no DRAM round-trip and no PE transposes are needed.  A 4-level DVE
max tree reduces k; all matmuls are bf16 with weights stationary.
"""

import numpy as np
import ml_dtypes

import concourse.bass as bass
import concourse.mybir as mybir
import concourse.tile as tile
from concourse import bacc
from concourse.bass_utils import run_bass_kernel_spmd

F32 = mybir.dt.float32
BF16 = mybir.dt.bfloat16
I16 = mybir.dt.int16

B, N, K, C = 8, 4096, 16, 256
P = 128
CK = C // P          # 2 channel chunks
NT = N // P          # 32 proj stripes
NO = 8               # node octants
U = N // NO          # 512 nodes per octant
G = 2048             # indices per gather
GPO = K * U // G     # 4 gathers per octant (4 k-slots each)
EPS = 1e-5

AF = mybir.ActivationFunctionType
ALU = mybir.AluOpType


def build_bass(n_cores: int = 8, reps: int = 1):
    nc = bacc.Bacc(
        "TRN2",
        target_bir_lowering=False,
        debug=False,
        enable_asserts=False,
        num_devices=n_cores,
        num_swdge_queues=4,
        dynamic_dma_scratch_size=32768,
    )

    xT = nc.dram_tensor("xT", [C, N], BF16, kind="ExternalInput").ap()
    knn_i = nc.dram_tensor("knn_i", [P, NO * GPO * (G // 16)], I16, kind="ExternalInput").ap()
    # packed bf16 weights: block (w_i, kc) = W^T[kc*128:(kc+1)*128, :] at
    # [:, (w_i*CK+kc)*C : +C]; order: wp, wx1, wx2, wm1, w1'', w2'
    wb = nc.dram_tensor("wb", [P, 6 * CK * C], BF16, kind="ExternalInput").ap()
    bias = nc.dram_tensor("bias", [P, 10], F32, kind="ExternalInput").ap()
    outT = nc.dram_tensor("outT", [C, N], BF16, kind="ExternalOutput").ap()

    with tile.TileContext(nc) as tc:
        for _ in range(reps):
            kernel_body(tc, xT, knn_i, wb, bias, outT)
    nc.compile()
    return nc


def kernel_body(tc, xT, knn_i, wb, bias, outT):
    nc = tc.nc

    with (
        tc.tile_pool(name="const", bufs=1) as cpool,
        tc.tile_pool(name="projp", bufs=1) as projp,
        tc.tile_pool(name="gat", bufs=1) as gat,
        tc.tile_pool(name="mtp", bufs=1) as mtp,
        tc.tile_pool(name="units", bufs=2) as up,
        tc.tile_pool(name="outp", bufs=2) as outp,
        tc.tile_pool(name="psP", bufs=2, space="PSUM") as psP,
        tc.tile_pool(name="psX", bufs=4, space="PSUM") as psX,
        tc.tile_pool(name="psH", bufs=2, space="PSUM") as psH,
    ):
        # ---- constants / inputs to SBUF (order = first-use order) ----
        wb_sb = cpool.tile([P, 6 * CK * C], BF16)
        nc.sync.dma_start(wb_sb[:, 0:CK * C], wb[:, 0:CK * C])          # wp first

        def wslice(w_i):
            return [wb_sb[:, (w_i * CK + kc) * C:(w_i * CK + kc + 1) * C]
                    for kc in range(CK)]

        wp_sb, wx1_sb, wx2_sb, wm1_sb, w1_sb, w2_sb = (wslice(i) for i in range(6))

        # xt[p, kc, n]: x^T in bf16, loaded per octant for early proj start
        xt = cpool.tile([P, CK, N], BF16)
        xTr = xT.rearrange("(kc p) n -> p kc n", p=P)
        for o in range(NO):
            nc.sync.dma_start(xt[:, :, o * U:(o + 1) * U],
                              xTr[:, :, o * U:(o + 1) * U])

        kidx = cpool.tile([P, NO * GPO, G // 16], I16)
        nc.sync.dma_start(kidx[:], knn_i[:].rearrange("p (g c) -> p g c", c=G // 16))

        nc.sync.dma_start(wb_sb[:, CK * C:], wb[:, CK * C:])

        bias_sb = cpool.tile([P, 10], F32)
        nc.sync.dma_start(bias_sb[:], bias[:])

        # ---- phase 1: proj, node-major bf16, stays in SBUF ----
        # proj_sb[p, t, c] = proj[node t*128+p, c]; two stripes share one PSUM
        # bank so each evac moves 512 elems; evacs alternate Act/DVE.
        proj_sb = projp.tile([P, NT, C], BF16)
        for tp in range(NT // 2):
            ps = psP.tile([P, 2, C], F32, name="ps_proj", tag="ps_proj")
            for half in range(2):
                t = 2 * tp + half
                nc.tensor.matmul(ps[:, half, :], lhsT=xt[:, 0, t * P:(t + 1) * P],
                                 rhs=wp_sb[0], start=half == 0 and True, stop=False,
                                 skip_group_check=True)
                nc.tensor.matmul(ps[:, half, :], lhsT=xt[:, 1, t * P:(t + 1) * P],
                                 rhs=wp_sb[1], start=False, stop=half == 1,
                                 skip_group_check=True)
            dst = proj_sb[:, 2 * tp:2 * tp + 2, :]
            if tp % 2 == 0:
                nc.scalar.activation(dst, ps[:], AF.Copy)
            else:
                nc.vector.tensor_copy(dst, ps[:])

        proj_flat = proj_sb[:].rearrange("p t c -> p (t c)")

        # ---- phases 2-4: per-octant pipeline ----
        # Three-stage software pipeline so no engine queue ever parks on a
        # cross-engine round-trip:
        #   A(o)  = gathers + max tree (DVE, one level on Pool)
        #   B1(o) = PE matmuls + Act evacs (x1/x2/h/a)
        #   B2(o) = DVE combines (dd, g2, out) + out DMA, one octant later
        mts = [None] * NO
        units = [None] * NO

        def phase_a(o):
            go = []
            for g in range(GPO):
                gt = gat.tile([P, CK, G], BF16, name=f"go{o}_{g}", tag="go", bufs=8)
                nc.gpsimd.dma_gather(
                    out_ap=gt[:],
                    in_ap=proj_flat,
                    idxs_ap=kidx[:, o * GPO + g, :],
                    num_idxs=G,
                    num_idxs_reg=G,
                    elem_size=C,
                    transpose=True,
                    single_packet=False,
                    sbuf_tokens_per_rank=P,
                    sbuf_free_dim_per_rank=C * 2,
                    queue_num=g % 4,
                )
                go.append(gt)

            t01 = mtp.tile([P, CK, G], BF16, name="t01", tag="t01", bufs=2)
            nc.vector.tensor_tensor(out=t01[:], in0=go[0][:], in1=go[1][:],
                                    op=ALU.max)
            t0123 = mtp.tile([P, CK, G], BF16, name="t0123", tag="t0123", bufs=2)
            nc.vector.tensor_tensor(out=t0123[:], in0=go[2][:], in1=go[3][:],
                                    op=ALU.max)
            nc.vector.tensor_tensor(out=t0123[:], in0=t0123[:], in1=t01[:],
                                    op=ALU.max)
            m3 = mtp.tile([P, CK, 2 * U], BF16, name="m3", tag="m3", bufs=2)
            nc.vector.tensor_tensor(out=m3[:], in0=t0123[:, :, :2 * U],
                                    in1=t0123[:, :, 2 * U:], op=ALU.max)
            mt = mtp.tile([P, CK, U], BF16, name="mt", tag="mt", bufs=3)
            nc.vector.tensor_tensor(out=mt[:], in0=m3[:, :, :U],
                                    in1=m3[:, :, U:], op=ALU.max)
            mts[o] = mt

        def phase_b1(o):
            mt = mts[o]
            x1_sb = up.tile([P, CK, U], BF16, name="x1", tag="x1", bufs=3)
            x2_sb = up.tile([P, CK, U], BF16, name="x2", tag="x2", bufs=3)
            h_sb = up.tile([P, CK, U], BF16, name="h", tag="h", bufs=3)
            a_sb = up.tile([P, CK, U], BF16, name="a", tag="a", bufs=3)
            units[o] = (x1_sb, x2_sb, a_sb)

            xo = [xt[:, kc, o * U:(o + 1) * U] for kc in range(CK)]

            for mc in range(CK):
                ps1 = psX.tile([P, U], F32, name="ps_x1", tag="ps_x1", bufs=2)
                for kc in range(CK):
                    nc.tensor.matmul(ps1[:], lhsT=wm1_sb[kc][:, mc * P:(mc + 1) * P],
                                     rhs=mt[:, kc, :], start=kc == 0, stop=False,
                                     skip_group_check=True)
                for kc in range(CK):
                    nc.tensor.matmul(ps1[:], lhsT=wx1_sb[kc][:, mc * P:(mc + 1) * P],
                                     rhs=xo[kc], start=False, stop=kc == CK - 1,
                                     skip_group_check=True)
                nc.scalar.activation(x1_sb[:, mc, :], ps1[:], AF.Identity,
                                     bias=bias_sb[:, 0 + mc:1 + mc], scale=1.0)
                ps2 = psX.tile([P, U], F32, name="ps_x2", tag="ps_x2", bufs=2)
                for kc in range(CK):
                    nc.tensor.matmul(ps2[:], lhsT=wx2_sb[kc][:, mc * P:(mc + 1) * P],
                                     rhs=xo[kc], start=kc == 0, stop=kc == CK - 1,
                                     skip_group_check=True)
                nc.scalar.activation(x2_sb[:, mc, :], ps2[:], AF.Identity,
                                     bias=bias_sb[:, 2 + mc:3 + mc], scale=1.0)

            # h = f@W1'' with f = X1 + X2 distributed into the matmul
            for mc in range(CK):
                psh = psH.tile([P, U], F32, name="ps_h", tag="ps_h", bufs=1)
                for kc in range(CK):
                    nc.tensor.matmul(psh[:], lhsT=w1_sb[kc][:, mc * P:(mc + 1) * P],
                                     rhs=x1_sb[:, kc, :], start=kc == 0, stop=False,
                                     skip_group_check=True)
                for kc in range(CK):
                    nc.tensor.matmul(psh[:], lhsT=w1_sb[kc][:, mc * P:(mc + 1) * P],
                                     rhs=x2_sb[:, kc, :], start=False,
                                     stop=kc == CK - 1, skip_group_check=True)
                nc.scalar.activation(h_sb[:, mc, :], psh[:], AF.Identity,
                                     bias=bias_sb[:, 4 + mc:5 + mc], scale=1.0)
            for mc in range(CK):
                psa = psH.tile([P, U], F32, name="ps_a", tag="ps_a", bufs=1)
                for kc in range(CK):
                    nc.tensor.matmul(psa[:], lhsT=w2_sb[kc][:, mc * P:(mc + 1) * P],
                                     rhs=h_sb[:, kc, :], start=kc == 0,
                                     stop=kc == CK - 1, skip_group_check=True)
                nc.scalar.activation(a_sb[:, mc, :], psa[:], AF.Sigmoid,
                                     bias=bias_sb[:, 6 + mc:7 + mc], scale=1.0)

        def phase_b2(o):
            x1_sb, x2_sb, a_sb = units[o]
            dd_sb = up.tile([P, CK, U], BF16, name="dd", tag="dd", bufs=2)
            g2_sb = up.tile([P, CK, U], BF16, name="g2", tag="g2", bufs=2)
            nc.vector.tensor_tensor(out=dd_sb[:], in0=x1_sb[:], in1=x2_sb[:],
                                    op=ALU.subtract)
            nc.vector.tensor_tensor(out=g2_sb[:], in0=a_sb[:], in1=dd_sb[:],
                                    op=ALU.mult)
            # out = (g2 + t_f) + X2, per out-channel chunk (t_f is per-partition)
            for mc in range(CK):
                ot = outp.tile([P, U], BF16, name="ot", tag="ot")
                nc.vector.scalar_tensor_tensor(
                    out=ot[:], in0=g2_sb[:, mc, :],
                    scalar=bias_sb[:, 8 + mc:9 + mc],
                    in1=x2_sb[:, mc, :], op0=ALU.add, op1=ALU.add)
                nc.sync.dma_start(outT[mc * P:(mc + 1) * P, o * U:(o + 1) * U], ot[:])

        phase_a(0)
        phase_a(1)
        phase_b1(0)
        for o in range(2, NO):
            phase_a(o)
            phase_b1(o - 1)
            phase_b2(o - 2)
        phase_b1(NO - 1)
        phase_b2(NO - 2)
        phase_b2(NO - 1)


# ---------------- host side ----------------

def _fold(proj_W, local_W, glob_W, aff_W1, aff_b1, aff_W2, aff_b2,
          bn_local, bn_glob, bn_aff1, bn_aff2, bn_final):
    f32 = np.float32

    def bn_st(p):
        p = np.asarray(p, f32)
        g, b, m, v = p
        s = g / np.sqrt(v + EPS)
        return s.astype(f32), (b - m * s).astype(f32)

    Wp = np.asarray(proj_W, f32)
    s_l, t_l = bn_st(bn_local)
    s_g, t_g = bn_st(bn_glob)
    s_1, t_1 = bn_st(bn_aff1)
    s_2, t_2 = bn_st(bn_aff2)
    s_f, t_f = bn_st(bn_final)

    Wlp = s_l[:, None] * np.asarray(local_W, f32)
    Wgp = s_g[:, None] * np.asarray(glob_W, f32)

    wp = Wp
    wx1 = -(s_f[:, None] * (Wlp @ Wp))
    wx2 = s_f[:, None] * Wgp
    wm1 = s_f[:, None] * Wlp
    w1 = (s_1[:, None] * np.asarray(aff_W1, f32)) * (1.0 / s_f)[None, :]
    w2 = s_2[:, None] * np.asarray(aff_W2, f32)

    out = np.zeros((P, 6 * CK * C), ml_dtypes.bfloat16)
    for w_i, m in enumerate((wp, wx1, wx2, wm1, w1, w2)):
        mT = m.T
        for kc in range(CK):
            out[:, (w_i * CK + kc) * C:(w_i * CK + kc + 1) * C] = \
                mT[kc * P:(kc + 1) * P, :].astype(ml_dtypes.bfloat16)

    b1c = s_f * t_l
    b2c = s_f * t_g
    bh = s_1 * np.asarray(aff_b1, f32) + t_1
    ba = s_2 * np.asarray(aff_b2, f32) + t_2
    bias = np.zeros((P, 10), f32)
    for j, tt in enumerate((b1c, b2c, bh, ba, t_f)):
        for mc in range(CK):
            bias[:, 2 * j + mc] = tt[mc * P:(mc + 1) * P]
    return {"wb": out, "bias": bias}


def _pack_idx(knn_b):
    """knn[b] [N, K] int -> [128, NO*GPO*(G//16)] int16 wrapped gather idxs.

    Per (octant o, gather g): 2048 idxs k-major (j = k'*U + i, k' in [0,4)),
    wrapped [16, G//16] with [w, c] = idx[c*16+w], replicated 8x over
    partition groups."""
    cols = []
    for o in range(NO):
        blk = knn_b[o * U:(o + 1) * U, :]          # [U, K]
        for g in range(GPO):
            arr = blk[:, 4 * g:4 * g + 4].T.reshape(-1)   # [4*U] k-major
            wrapped = arr.reshape(G // 16, 16).T           # [16, G//16]
            cols.append(np.tile(wrapped, (8, 1)))          # [128, G//16]
    return np.concatenate(cols, axis=1).astype(np.int16)


_NC_CACHE = {}


def _get_nc():
    if "nc" not in _NC_CACHE:
        _NC_CACHE["nc"] = build_bass(B)
    return _NC_CACHE["nc"]


def kernel(**inputs) -> np.ndarray:
    x = np.asarray(inputs["x"], np.float32)                 # [B,N,C]
    knn = np.asarray(inputs["knn"]).astype(np.int64)        # [B,N,K]
    w = _fold(
        inputs["proj_W"], inputs["local_W"], inputs["glob_W"],
        inputs["aff_W1"], inputs["aff_b1"], inputs["aff_W2"], inputs["aff_b2"],
        inputs["bn_local"], inputs["bn_glob"], inputs["bn_aff1"],
        inputs["bn_aff2"], inputs["bn_final"],
    )

    nc = _get_nc()
    in_maps = []
    for b in range(B):
        m = {"xT": np.ascontiguousarray(x[b].T).astype(ml_dtypes.bfloat16),
             "knn_i": _pack_idx(knn[b])}
        m.update(w)
        in_maps.append(m)

    res = run_bass_kernel_spmd(nc, in_maps, core_ids=list(range(B)))
    out = np.stack([res.results[b]["outT"].astype(np.float32).T
                    for b in range(B)])
    return out


if __name__ == "__main__":
    nc = build_bass(1)
    print("built OK")
